# revision 2
# baseline (speedup 1.0000x reference)
"""AdaptiveGridMerger Trainium2 kernel.

Math: the reference scatters x[b,c,:] into a flat 8x8 grid with bilinear
(4-corner) weights from positions[b,c,:], then matmuls grid_weights
GW [270,64]. The scatter matrix S_b [64,306] (column c = the bilinear
hat weights of channel c) is tiny and depends only on positions, so it
is built on the HOST. The tail output rows 256:270 are folded into it:
  st78[c, 0:64]  = S_b[:, c]
  st78[c, 64:78] = (S_b.T @ GW[256:270].T)[c]   (Wtail fold)
so mm1 (lhsT=st78) produces gv[0:64] = S@x AND gv[64:78] = out[256:270]
in one pass. mm2 (lhsT=GW[0:256].T) produces out[0:256] from gv[0:64].

Device schedule (per core, 2 batches):
- READS all ride the ACT HWDGE ring (nc.scalar.dma_start), queued at
  t=0 in consumption order: stgw pack, x-tail (2x[50,2048] at partition
  offsets 0/64 -> complementary SDMA engine sets, so they drain in
  parallel), x[0:128], x[128:256] in column chunks for quarter-gating.
- WRITES all ride the SP HWDGE ring (nc.sync.dma_start) so the output
  stream overlaps the input stream instead of draining serially after
  it; SDMA engines round-robin between the two rings.
- PE: mm1 accumulates gv quarters ([78,1024] fp32, 2 PSUM banks each)
  in data-arrival order (tail, ch0, ch1); as each quarter stops it is
  evac'd (DVE/ACT alternating) to bf16 gvt and mm2 + out-evac + write
  follow per quarter, so writes start ~8us into the kernel.
- 8 warm-up spin matmuls burn the ~3.4us HAM cold window during the
  DMA lead-in; no other filler matmuls (PE has large slack vs DMA).

PSUM: one pool of 4 x [128,1024] f32 slots = exactly 8 banks; mm1
holds up to 4 gv quarter accumulators, mm2's o_ps tiles rotate through
the freed slots.

Sharding: data-parallel over batch, 2 batches per core.
"""

import numpy as np

import concourse.bass as bass
import concourse.bacc as bacc
import concourse.mybir as mybir
from concourse import tile
from concourse.bass_utils import run_bass_kernel_spmd

B, C, T = 16, 306, 4096
M, G, GS = 270, 64, 8
N_CORES = 8
BL = B // N_CORES  # batches per core

W78 = G + 14          # st block width: 64 grid cols + 14 folded tail cols
XC = T // 2           # tail-tile column count (T half)
STB = 3 * W78         # st cols per batch (ch0 block, ch1 block, tail block)
GWC = BL * STB        # gw halves base col in stgw pack
SW = GWC + 2 * 128    # stgw pack width: 468 + 256 = 724
T_PS = 512
TQ = 1024             # pipeline quarter width
N_SPIN = 8

MM_DTYPE = mybir.dt.bfloat16
NP_MM = mybir.dt.np(MM_DTYPE)
FP32 = mybir.dt.float32


def build_nc():
    nc = bacc.Bacc()
    stgw_ext = nc.declare_dram_parameter("stgw", [128, SW], MM_DTYPE, isOutput=False)
    xt_ext = nc.declare_dram_parameter("xt", [BL, 2, 50, XC], MM_DTYPE, isOutput=False)
    xm_ext = nc.declare_dram_parameter("xm", [BL, 2, 128, T], MM_DTYPE, isOutput=False)
    out_ext = nc.declare_dram_parameter("out", [BL, M, T], MM_DTYPE, isOutput=True)

    with tile.TileContext(nc) as tc:
        with (
            tc.tile_pool(name="const", bufs=1) as constp,
            tc.tile_pool(name="xp", bufs=1) as xp,
            tc.tile_pool(name="gvt", bufs=2) as gvtp,
            tc.tile_pool(name="op", bufs=6) as outp,
            tc.tile_pool(name="ps", bufs=4, space=bass.MemorySpace.PSUM) as psp,
        ):
            # PE clock pre-ramp: burn the HAM cold window on dummy work
            # while the first reads stream in.
            dummy = constp.tile([128, T_PS], MM_DTYPE, tag="dummy")
            nc.vector.memset(dummy[:], 0.0)
            spin_ps = psp.tile([128, TQ], FP32, tag="pb", name="spin_ps")
            for _ in range(N_SPIN):
                nc.tensor.matmul(
                    spin_ps[:, :T_PS], dummy[:, :128], dummy[:], start=True, stop=True
                )

            # ---- reads: all on the ACT ring, in consumption order
            stgw = constp.tile([128, SW], MM_DTYPE, tag="stgw")
            nc.scalar.dma_start(out=stgw[:], in_=stgw_ext[:])
            xt, xc0, xc1 = {}, {}, {}
            for b in range(BL):
                xt[b] = xp.tile([128, XC], MM_DTYPE, tag=f"xt{b}", name=f"xt{b}")
                xc0[b] = xp.tile([128, T], MM_DTYPE, tag=f"xc0_{b}", name=f"xc0_{b}")
                xc1[b] = xp.tile([128, T], MM_DTYPE, tag=f"xc1_{b}", name=f"xc1_{b}")
            for b in range(BL):
                for tt in range(2):
                    p0 = 64 * tt
                    nc.scalar.dma_start(out=xt[b][p0 : p0 + 50, :], in_=xt_ext[b, tt])
                nc.scalar.dma_start(out=xc0[b][:], in_=xm_ext[b, 0])
                # ch1 half arrives last -> split for quarter-gating
                nc.scalar.dma_start(
                    out=xc1[b][:, 0:2048], in_=xm_ext[b, 1, :, 0:2048]
                )
                nc.scalar.dma_start(
                    out=xc1[b][:, 2048:3072], in_=xm_ext[b, 1, :, 2048:3072]
                )
                nc.scalar.dma_start(
                    out=xc1[b][:, 3072:4096], in_=xm_ext[b, 1, :, 3072:4096]
                )

            k_state = {"k": 0}

            def evac(dst, src):
                if k_state["k"] % 2 == 0:
                    nc.vector.tensor_copy(dst, src)
                else:
                    nc.scalar.copy(dst, src)
                k_state["k"] += 1

            gvts = {}
            for b in range(BL):
                gvts[b] = gvtp.tile([W78, T], MM_DTYPE, tag="gvt", name=f"gvt{b}")

            gv = {}  # (b, q) -> live psum quarter accumulator

            def mm1(b, q, which, start, stop):
                # which 0: ch0 (K=128), 1: ch1 (K=128), 2: tail (K=50)
                if (b, q) not in gv:
                    gv[(b, q)] = psp.tile([128, TQ], FP32, tag="pb", name=f"gv{b}_{q}")
                for s in range(2):
                    dst = gv[(b, q)][:W78, s * T_PS : (s + 1) * T_PS]
                    if which == 2:
                        p0 = 64 if q >= 2 else 0
                        c0 = (q % 2) * TQ + s * T_PS
                        lhs = stgw[p0 : p0 + 50, b * STB + 2 * W78 : b * STB + 3 * W78]
                        rhs = xt[b][p0 : p0 + 50, c0 : c0 + T_PS]
                    else:
                        lhs = stgw[0:128, b * STB + which * W78 : b * STB + (which + 1) * W78]
                        src = xc0[b] if which == 0 else xc1[b]
                        c0 = q * TQ + s * T_PS
                        rhs = src[:, c0 : c0 + T_PS]
                    nc.tensor.matmul(
                        dst, lhs, rhs, start=start, stop=stop, skip_group_check=True
                    )

            def evac_gvt(b, q):
                evac(gvts[b][:W78, q * TQ : (q + 1) * TQ], gv[(b, q)][:W78])
                del gv[(b, q)]

            def mm2_quarter(b, q):
                for mi in range(2):
                    o_ps = psp.tile([128, TQ], FP32, tag="pb", name=f"o{b}_{q}_{mi}")
                    for s in range(2):
                        c0 = q * TQ + s * T_PS
                        nc.tensor.matmul(
                            o_ps[:, s * T_PS : (s + 1) * T_PS],
                            stgw[0:G, GWC + mi * 128 : GWC + (mi + 1) * 128],
                            gvts[b][0:G, c0 : c0 + T_PS],
                            start=True, stop=True, skip_group_check=True,
                        )
                    o_sb = outp.tile([128, TQ], MM_DTYPE, tag="o", name=f"ot{b}_{q}_{mi}")
                    evac(o_sb[:], o_ps[:])
                    nc.sync.dma_start(
                        out=out_ext[b, mi * 128 : (mi + 1) * 128, q * TQ : (q + 1) * TQ],
                        in_=o_sb[:],
                    )

            for b in range(BL):
                # mm1 in data-arrival order; quarters complete as ch1 cols land
                for q in range(4):
                    mm1(b, q, 2, True, False)
                for q in range(4):
                    mm1(b, q, 0, False, False)
                for q in range(2):
                    mm1(b, q, 1, False, True)
                evac_gvt(b, 0)
                evac_gvt(b, 1)
                for q in range(2, 4):
                    mm1(b, q, 1, False, True)
                evac_gvt(b, 2)
                evac_gvt(b, 3)
                for q in range(4):
                    mm2_quarter(b, q)
                # folded tail rows out[256:270] ride gvt rows 64:78
                nc.sync.dma_start(out=out_ext[b, 256:M, :], in_=gvts[b][G:W78, :])
    nc.compile()
    return nc


def _host_st(positions, grid_weights):
    """st78 [B, C, 78] f32: bilinear hat weights + folded tail rows."""
    gp = (positions.astype(np.float32) + 1.0) * (GS / 2.0)  # [B, C, 2]
    i = np.arange(GS, dtype=np.float32)
    wy = np.maximum(0.0, 1.0 - np.abs(i[None, None, :] - gp[:, :, 0:1]))
    wx = np.maximum(0.0, 1.0 - np.abs(i[None, None, :] - gp[:, :, 1:2]))
    s = (wy[:, :, :, None] * wx[:, :, None, :]).reshape(B, C, G)
    wtail = s @ grid_weights[256:M].T.astype(np.float32)  # [B, C, 14]
    return np.concatenate([s, wtail], axis=2)


def make_in_maps(x, positions, grid_weights):
    st78 = _host_st(positions, grid_weights)
    gw = np.ascontiguousarray(grid_weights[:256].T).astype(np.float32)  # [64, 256]
    x_mm = x.astype(NP_MM)
    in_maps = []
    for i in range(N_CORES):
        sl = slice(i * BL, (i + 1) * BL)
        stgw_pack = np.zeros((128, SW), dtype=np.float32)
        xt_pack = np.empty((BL, 2, 50, XC), dtype=NP_MM)
        for b2 in range(BL):
            gb = i * BL + b2
            c0 = b2 * STB
            stgw_pack[:, c0 : c0 + W78] = st78[gb, 0:128]
            stgw_pack[:, c0 + W78 : c0 + 2 * W78] = st78[gb, 128:256]
            stgw_pack[0:50, c0 + 2 * W78 : c0 + 3 * W78] = st78[gb, 256:C]
            stgw_pack[64:114, c0 + 2 * W78 : c0 + 3 * W78] = st78[gb, 256:C]
            stgw_pack[0:G, GWC + b2 * 128 : GWC + (b2 + 1) * 128] = gw[
                :, b2 * 128 : (b2 + 1) * 128
            ]
            xt_pack[b2] = np.moveaxis(
                x_mm[gb, 256:C].reshape(50, 2, XC), 1, 0
            )
        in_maps.append(
            {
                "stgw": stgw_pack.astype(NP_MM),
                "xt": xt_pack,
                "xm": np.ascontiguousarray(x_mm[sl, 0:256]).reshape(BL, 2, 128, T),
            }
        )
    return in_maps


_NC_CACHE = None


def kernel(x, positions, grid_weights):
    global _NC_CACHE
    if _NC_CACHE is None:
        _NC_CACHE = build_nc()
    nc = _NC_CACHE
    in_maps = make_in_maps(x, positions, grid_weights)
    res = run_bass_kernel_spmd(nc, in_maps, core_ids=list(range(N_CORES)))
    out = np.concatenate([r["out"] for r in res.results], axis=0)
    return np.asarray(out, dtype=np.float32)


if __name__ == "__main__":
    xs = np.random.randn(B, C, T).astype(np.float32)
    ps = np.random.uniform(-1, 0.74, (B, C, 2)).astype(np.float32)
    gw = np.random.randn(M, G).astype(np.float32)
    out = kernel(xs, ps, gw)
    print(out.shape, out.dtype)


# revision 3
# speedup vs baseline: 1.1498x; 1.1498x over previous
"""AdaptiveGridMerger Trainium2 kernel.

Math: the reference scatters x[b,c,:] into a flat 8x8 grid with bilinear
(4-corner) weights from positions[b,c,:], then matmuls grid_weights
GW [270,64]. The scatter matrix S_b [64,306] (column c = the bilinear
hat weights of channel c) is tiny and depends only on positions, so it
is built on the HOST. The tail output rows 256:270 are folded into it:
  st78[c, 0:64]  = S_b[:, c]
  st78[c, 64:78] = (S_b.T @ GW[256:270].T)[c]   (Wtail fold)
so mm1 (lhsT=st78) produces gv[0:64] = S@x AND gv[64:78] = out[256:270]
in one pass. mm2 (lhsT=GW[0:256].T) produces out[0:256] from gv[0:64].

Engine budget (the binding constraints, measured):
- dma_start occupies the ISSUING engine ~0.6us + ~0.7us/MB (descgen),
  so reads ride SP (otherwise idle) and writes ride GpSimd/SWDGE
  (otherwise idle); DVE+ACT are reserved for PSUM->SBUF evac copies
  (~1.1-1.2us per [128,1024]).
- One busy DMA ring saturates HBM (~360-400 GB/s observed), so the
  goal is simply: some ring always has ready work. Reads are all
  queued at t=0; writes flow per T-quarter as soon as produced.
- st/gw ride inside the first xc0 read (extra columns, full-width
  lines); the 50 tail channels are packed [50+50 rows, 2048] into a
  padded [128,2048] tile so the read uses all 16 SDMA engines.

PE: mm1 accumulates gv quarters ([78,1024] f32, 2 PSUM banks) with
group order xc0(start) -> tail -> xc1(stop) matching read arrival;
mm2+evac+write follow per quarter; batch-1 mm1 groups interleave with
batch-0 mm2 to keep PE dense (HAM warm). 8 spin matmuls burn the
~3.4us HAM cold window during the DMA lead-in.

Sharding: data-parallel over batch, 2 batches per core.
"""

import numpy as np

import concourse.bass as bass
import concourse.bacc as bacc
import concourse.mybir as mybir
from concourse import tile
from concourse.bass_utils import run_bass_kernel_spmd

B, C, T = 16, 306, 4096
M, G, GS = 270, 64, 8
N_CORES = 8
BL = B // N_CORES  # batches per core

W78 = G + 14          # st block width: 64 grid cols + 14 folded tail cols
XC = T // 2
STB = 3 * W78         # st cols per batch (ch0, ch1, tail blocks)
SC = T                # st base col inside the xa pack
GWC = SC + BL * STB   # gw halves base col
XA = GWC + 2 * 128    # xa pack width: 4096 + 468 + 256 = 4820
T_PS = 512
TQ = 1024
N_SPIN = 8

MM_DTYPE = mybir.dt.bfloat16
NP_MM = mybir.dt.np(MM_DTYPE)
FP32 = mybir.dt.float32


def build_nc():
    nc = bacc.Bacc()
    xa_ext = nc.declare_dram_parameter("xa", [128, XA], MM_DTYPE, isOutput=False)
    xd_ext = nc.declare_dram_parameter("xd", [128, T], MM_DTYPE, isOutput=False)
    xb_ext = nc.declare_dram_parameter("xb", [BL, 128, XC], MM_DTYPE, isOutput=False)
    xc_ext = nc.declare_dram_parameter("xc", [BL, 128, T], MM_DTYPE, isOutput=False)
    out_ext = nc.declare_dram_parameter("out", [BL, M, T], MM_DTYPE, isOutput=True)

    with tile.TileContext(nc) as tc:
        with (
            tc.tile_pool(name="const", bufs=1) as constp,
            tc.tile_pool(name="xp", bufs=1) as xp,
            tc.tile_pool(name="gvt", bufs=2) as gvtp,
            tc.tile_pool(name="op", bufs=6) as outp,
            tc.tile_pool(name="ps", bufs=4, space=bass.MemorySpace.PSUM) as psp,
        ):
            # PE clock pre-ramp: burn the HAM cold window on dummy work
            # while the first reads stream in.
            dummy = constp.tile([128, T_PS], MM_DTYPE, tag="dummy")
            nc.vector.memset(dummy[:], 0.0)
            spin_ps = psp.tile([128, TQ], FP32, tag="pb", name="spin_ps")
            for _ in range(N_SPIN):
                nc.tensor.matmul(
                    spin_ps[:, :T_PS], dummy[:, :128], dummy[:], start=True, stop=True
                )

            # ---- reads: all on the SP ring, in consumption order
            xa = xp.tile([128, XA], MM_DTYPE, tag="xa", name="xa")
            xd = xp.tile([128, T], MM_DTYPE, tag="xd", name="xd")
            xb, xc = {}, {}
            for b in range(BL):
                xb[b] = xp.tile([128, XC], MM_DTYPE, tag=f"xb{b}", name=f"xb{b}")
                xc[b] = xp.tile([128, T], MM_DTYPE, tag=f"xc{b}", name=f"xc{b}")
            nc.sync.dma_start(out=xa[:], in_=xa_ext[:])
            nc.sync.dma_start(out=xb[0][:], in_=xb_ext[0])
            nc.sync.dma_start(out=xc[0][:, 0:2048], in_=xc_ext[0, :, 0:2048])
            nc.sync.dma_start(out=xc[0][:, 2048:3072], in_=xc_ext[0, :, 2048:3072])
            nc.sync.dma_start(out=xc[0][:, 3072:4096], in_=xc_ext[0, :, 3072:4096])
            nc.sync.dma_start(out=xd[:], in_=xd_ext[:])
            nc.sync.dma_start(out=xb[1][:], in_=xb_ext[1])
            nc.sync.dma_start(out=xc[1][:, 0:2048], in_=xc_ext[1, :, 0:2048])
            nc.sync.dma_start(out=xc[1][:, 2048:3072], in_=xc_ext[1, :, 2048:3072])
            nc.sync.dma_start(out=xc[1][:, 3072:4096], in_=xc_ext[1, :, 3072:4096])

            k_state = {"k": 0}

            def evac(dst, src):
                if k_state["k"] % 2 == 0:
                    nc.vector.tensor_copy(dst, src)
                else:
                    nc.scalar.copy(dst, src)
                k_state["k"] += 1

            gvts = {}
            for b in range(BL):
                gvts[b] = gvtp.tile([W78, T], MM_DTYPE, tag="gvt", name=f"gvt{b}")

            gv = {}  # (b, q) -> live psum quarter accumulator

            def mm1(b, q, which, start, stop):
                # which 0: ch0 (K=128), 1: ch1 (K=128), 2: tail (K=50)
                if (b, q) not in gv:
                    gv[(b, q)] = psp.tile([128, TQ], FP32, tag="pb", name=f"gv{b}_{q}")
                for s in range(2):
                    dst = gv[(b, q)][:W78, s * T_PS : (s + 1) * T_PS]
                    if which == 2:
                        p0 = 64 if q >= 2 else 0
                        c0 = (q % 2) * TQ + s * T_PS
                        lhs = xa[p0 : p0 + 50, SC + b * STB + 2 * W78 : SC + b * STB + 3 * W78]
                        rhs = xb[b][p0 : p0 + 50, c0 : c0 + T_PS]
                    else:
                        lhs = xa[0:128, SC + b * STB + which * W78 : SC + b * STB + (which + 1) * W78]
                        src = (xa if b == 0 else xd) if which == 0 else xc[b]
                        c0 = q * TQ + s * T_PS
                        rhs = src[:, c0 : c0 + T_PS]
                    nc.tensor.matmul(
                        dst, lhs, rhs, start=start, stop=stop, skip_group_check=True
                    )

            def evac_gvt(b, q):
                evac(gvts[b][:W78, q * TQ : (q + 1) * TQ], gv[(b, q)][:W78])
                del gv[(b, q)]

            out_sb = {}  # (b, mi, h) -> [128, 2048] staging tile

            def mm2_quarter(b, q):
                h = q // 2
                for mi in range(2):
                    o_ps = psp.tile([128, TQ], FP32, tag="pb", name=f"o{b}_{q}_{mi}")
                    for s in range(2):
                        c0 = q * TQ + s * T_PS
                        nc.tensor.matmul(
                            o_ps[:, s * T_PS : (s + 1) * T_PS],
                            xa[0:G, GWC + mi * 128 : GWC + (mi + 1) * 128],
                            gvts[b][0:G, c0 : c0 + T_PS],
                            start=True, stop=True, skip_group_check=True,
                        )
                    if (b, mi, h) not in out_sb:
                        out_sb[(b, mi, h)] = outp.tile(
                            [128, 2 * TQ], MM_DTYPE, tag="o", name=f"ot{b}_{mi}_{h}"
                        )
                    evac(out_sb[(b, mi, h)][:, (q % 2) * TQ : (q % 2 + 1) * TQ], o_ps[:])

            def write_half(b, h):
                for mi in range(2):
                    nc.gpsimd.dma_start(
                        out=out_ext[b, mi * 128 : (mi + 1) * 128, h * XC : (h + 1) * XC],
                        in_=out_sb[(b, mi, h)][:],
                    )
                    del out_sb[(b, mi, h)]

            # ---- PE program (interleaved for density / PSUM rotation)
            for q in range(4):
                mm1(0, q, 0, True, False)
            for q in range(4):
                mm1(0, q, 2, False, False)
            for q in range(2):
                mm1(0, q, 1, False, True)
            evac_gvt(0, 0)
            evac_gvt(0, 1)
            mm2_quarter(0, 0)
            for q in range(2, 4):
                mm1(0, q, 1, False, True)
            evac_gvt(0, 2)
            evac_gvt(0, 3)
            mm2_quarter(0, 1)
            write_half(0, 0)
            for q in range(2):
                mm1(1, q, 0, True, False)
            mm2_quarter(0, 2)
            for q in range(2, 4):
                mm1(1, q, 0, True, False)
            mm2_quarter(0, 3)
            write_half(0, 1)
            nc.gpsimd.dma_start(out=out_ext[0, 256:M, :], in_=gvts[0][G:W78, :])
            for q in range(4):
                mm1(1, q, 2, False, False)
            for q in range(2):
                mm1(1, q, 1, False, True)
            evac_gvt(1, 0)
            evac_gvt(1, 1)
            mm2_quarter(1, 0)
            for q in range(2, 4):
                mm1(1, q, 1, False, True)
            evac_gvt(1, 2)
            evac_gvt(1, 3)
            mm2_quarter(1, 1)
            write_half(1, 0)
            mm2_quarter(1, 2)
            mm2_quarter(1, 3)
            write_half(1, 1)
            nc.gpsimd.dma_start(out=out_ext[1, 256:M, :], in_=gvts[1][G:W78, :])
    nc.compile()
    return nc


def _host_st(positions, grid_weights):
    """st78 [B, C, 78] f32: bilinear hat weights + folded tail rows."""
    gp = (positions.astype(np.float32) + 1.0) * (GS / 2.0)  # [B, C, 2]
    i = np.arange(GS, dtype=np.float32)
    wy = np.maximum(0.0, 1.0 - np.abs(i[None, None, :] - gp[:, :, 0:1]))
    wx = np.maximum(0.0, 1.0 - np.abs(i[None, None, :] - gp[:, :, 1:2]))
    s = (wy[:, :, :, None] * wx[:, :, None, :]).reshape(B, C, G)
    wtail = s @ grid_weights[256:M].T.astype(np.float32)  # [B, C, 14]
    return np.concatenate([s, wtail], axis=2)


def make_in_maps(x, positions, grid_weights):
    st78 = _host_st(positions, grid_weights)
    gw = np.ascontiguousarray(grid_weights[:256].T).astype(np.float32)  # [64, 256]
    x_mm = x.astype(NP_MM)
    in_maps = []
    for i in range(N_CORES):
        xa_pack = np.zeros((128, XA), dtype=np.float32)
        xb_pack = np.zeros((BL, 128, XC), dtype=NP_MM)
        for b2 in range(BL):
            gb = i * BL + b2
            c0 = SC + b2 * STB
            xa_pack[:, c0 : c0 + W78] = st78[gb, 0:128]
            xa_pack[:, c0 + W78 : c0 + 2 * W78] = st78[gb, 128:256]
            xa_pack[0:50, c0 + 2 * W78 : c0 + 3 * W78] = st78[gb, 256:C]
            xa_pack[64:114, c0 + 2 * W78 : c0 + 3 * W78] = st78[gb, 256:C]
            xa_pack[0:G, GWC + b2 * 128 : GWC + (b2 + 1) * 128] = gw[
                :, b2 * 128 : (b2 + 1) * 128
            ]
            xtail = x_mm[gb, 256:C].reshape(50, 2, XC)
            xb_pack[b2, 0:50] = xtail[:, 0]
            xb_pack[b2, 64:114] = xtail[:, 1]
        g0 = i * BL
        xa_pack[:, 0:T] = x_mm[g0, 0:128]
        in_maps.append(
            {
                "xa": xa_pack.astype(NP_MM),
                "xd": np.ascontiguousarray(x_mm[g0 + 1, 0:128]),
                "xb": xb_pack,
                "xc": np.ascontiguousarray(x_mm[g0 : g0 + BL, 128:256]),
                # row blocks: ch0 = chans 0:128, ch1 = chans 128:256
            }
        )
    return in_maps


_NC_CACHE = None


def kernel(x, positions, grid_weights):
    global _NC_CACHE
    if _NC_CACHE is None:
        _NC_CACHE = build_nc()
    nc = _NC_CACHE
    in_maps = make_in_maps(x, positions, grid_weights)
    res = run_bass_kernel_spmd(nc, in_maps, core_ids=list(range(N_CORES)))
    out = np.concatenate([r["out"] for r in res.results], axis=0)
    return np.asarray(out, dtype=np.float32)


if __name__ == "__main__":
    xs = np.random.randn(B, C, T).astype(np.float32)
    ps = np.random.uniform(-1, 0.74, (B, C, 2)).astype(np.float32)
    gw = np.random.randn(M, G).astype(np.float32)
    out = kernel(xs, ps, gw)
    print(out.shape, out.dtype)
